# revision 12
# baseline (speedup 1.0000x reference)
"""Expert-parallel Conv1dBlock (Conv1d + GroupNorm + Mish) for Trainium2.

Strategy: 8 experts -> 8 NeuronCores. The host routes each sample to its
expert's core (MoE dispatch done as the sharding step), pads every core to a
common sample count, and each core runs an identical Bass/Tile program:

  - conv1d as matmuls over (Cin x K) contraction in a single float32r pass
    (11-bit mantissa, ~5e-4 rel err -- well inside the 2e-2 gate)
  - GroupNorm stats ride free on ACT/Pool accumulators; cross-partition
    group reduction via tiny 0/1 matmuls
  - rsqrt(var+eps) via fast-inverse-sqrt bit trick + Newton steps on DVE
  - Mish spread across ACT/Pool/DVE:
      w  = Exp(scol*y + tcol)          ACT (per-partition affine APs)
      q  = Square(w, bias=1) = (w+1)^2 ACT
      a  = q + 1                       Pool
      ra = recip_approx_fast(a)        DVE (1 op, ~51 ULP)
      rr = -2*ra + 1                   Pool
      out= (scol*y + tcol) * rr        DVE affine_mul_reduce (1 op)
"""

import sys

if "/opt/trn_rl_repo" not in sys.path:
    sys.path.insert(0, "/opt/trn_rl_repo")

import numpy as np

B, C, T = 512, 256, 256
E, KS, G = 8, 5, 8
EPS = 1e-5
HALF = C // 2  # 128, channels per partition block
GRP = C // G  # 32 channels per group
TP = T + 4  # padded time axis (2 halo columns each side)

PW = 8          # pairs per wave (stats batch)
TRACE = False   # set True (module-global) to run with NTFF profiling
LAST_EXEC_NS = None
LAST_RESULTS = None

_prog_cache = {}


def _install_trace_hook():
    import types

    if "antenv.axon_hooks" not in sys.modules:
        mod = types.ModuleType("antenv.axon_hooks")
        holder = [None]
        mod.set_axon_ntff_profile_hook = lambda h: holder.__setitem__(0, h)
        mod.get_axon_ntff_profile_hook = lambda: holder[0]
        sys.modules["antenv.axon_hooks"] = mod
        import antenv

        antenv.axon_hooks = mod
        from trn_agent_boot.trn_boot import _ntff_profile_via_ctypes

        mod.set_axon_ntff_profile_hook(
            _ntff_profile_via_ctypes("/opt/axon/libaxon_pjrt.so")
        )
    from concourse import bass_utils

    bass_utils.upload_artifacts = lambda tmpdir: f"local:{tmpdir}"


def _build_program(NP):
    import concourse.bacc as bacc
    import concourse.tile as tile
    from concourse import mybir

    dt = mybir.dt
    alu = mybir.AluOpType
    act = mybir.ActivationFunctionType

    nc = bacc.Bacc(None, target_bir_lowering=False)

    xh = nc.dram_tensor("xh", [NP, 2, HALF, 2 * TP], dt.float32r, kind="ExternalInput")
    # weights laid out [co_blk, ci_blk, ci, k, co]
    wh = nc.dram_tensor("wh", [2, 2, HALF, KS, HALF], dt.float32r, kind="ExternalInput")
    bias2 = nc.dram_tensor("bias2", [HALF, 2], dt.float32, kind="ExternalInput")
    gamma2 = nc.dram_tensor("gamma2", [HALF, 2], dt.float32, kind="ExternalInput")
    beta2 = nc.dram_tensor("beta2", [HALF, 2], dt.float32, kind="ExternalInput")
    gmat = nc.dram_tensor("gmat", [2, HALF, HALF], dt.float32r, kind="ExternalInput")
    amat = nc.dram_tensor("amat", [2, HALF, HALF], dt.float32r, kind="ExternalInput")
    yo = nc.dram_tensor("yo", [NP, 2, HALF, 2, T], dt.float32, kind="ExternalOutput")

    n_waves = (NP + PW - 1) // PW
    inv_n = 1.0 / (GRP * T)

    with tile.TileContext(nc) as tc:
        import contextlib

        with contextlib.ExitStack() as ctx:
            singles = ctx.enter_context(tc.tile_pool(name="singles", bufs=1))
            xpool = ctx.enter_context(tc.tile_pool(name="xpool", bufs=3))
            cpsum = ctx.enter_context(tc.tile_pool(name="cpsum", bufs=3, space="PSUM"))
            ybpool = ctx.enter_context(tc.tile_pool(name="ybpool", bufs=2 * PW + 2))
            y2pool = ctx.enter_context(tc.tile_pool(name="y2pool", bufs=2))
            swpool = ctx.enter_context(tc.tile_pool(name="swpool", bufs=2))
            spsum = ctx.enter_context(tc.tile_pool(name="spsum", bufs=1, space="PSUM"))
            bpsum = ctx.enter_context(tc.tile_pool(name="bpsum", bufs=1, space="PSUM"))
            statp = ctx.enter_context(tc.tile_pool(name="statp", bufs=2))
            stp = ctx.enter_context(tc.tile_pool(name="stp", bufs=2))
            mpool = ctx.enter_context(tc.tile_pool(name="mpool", bufs=3))
            scrp = ctx.enter_context(tc.tile_pool(name="scrp", bufs=4))
            otpool = ctx.enter_context(tc.tile_pool(name="otpool", bufs=4))

            # ---- constants / weights resident in SBUF ----
            # split per (cb, cib, k) so the first matmul's slice lands fast
            wsb_h = singles.tile([HALF, 2, 2, KS, HALF], dt.float32r)
            for cb in range(2):
                for cib in range(2):
                    for k in range(KS):
                        nc.sync.dma_start(out=wsb_h[:, cb, cib, k, :],
                                          in_=wh[cb, cib, :, k, :])
            bias_s = singles.tile([HALF, 2], dt.float32)
            nc.sync.dma_start(out=bias_s, in_=bias2[:, :])
            gamma_s = singles.tile([HALF, 2], dt.float32)
            nc.sync.dma_start(out=gamma_s, in_=gamma2[:, :])
            beta_s = singles.tile([HALF, 2], dt.float32)
            nc.sync.dma_start(out=beta_s, in_=beta2[:, :])
            gmat_s = singles.tile([HALF, 2, HALF], dt.float32r)
            nc.sync.dma_start(out=gmat_s, in_=gmat.rearrange("c p g -> p c g"))
            amat_s = singles.tile([HALF, 2, HALF], dt.float32r)
            nc.sync.dma_start(out=amat_s, in_=amat.rearrange("c g p -> g c p"))
            magic_s = singles.tile([G, 2 * PW], dt.int32)
            nc.vector.memset(magic_s, 0x5F3759DF)

            # state carried between waves for the deferred stats+Mish pass
            prev_wave = None  # (list of (p, ybs, iw_base), swsum, swsq, nw2)

            def emit_mish(wave_state):
                # mish(z) = z * tanh(softplus(z)) = z * (1 - 2/((1+e^z)^2+1))
                # with z = scol*y + tcol. GpSimd only runs dual-op
                # tensor_scalar (MULTIPLY,ADD) forms -- single-op/bypass forms
                # are ~10x slower on that engine.
                pairs, scols, tcols = wave_state
                for (p, ybs, iw0) in pairs:
                    for cb in range(2):
                        zt = mpool.tile([HALF, 2, T], dt.float32, name=f"zt{cb}",
                                        tag=f"zt{cb}")
                        for s in range(2):
                            iw = iw0 + s
                            nc.gpsimd.tensor_scalar(
                                out=zt[:, s, :], in0=ybs[cb][:, s, :],
                                scalar1=scols[cb][:, iw:iw + 1],
                                scalar2=tcols[cb][:, iw:iw + 1],
                                op0=alu.mult, op1=alu.add)
                        w = mpool.tile([HALF, 2, T], dt.float32, name=f"w{cb}",
                                       tag=f"w{cb}")
                        nc.scalar.activation(out=w, in_=zt, func=act.Exp)
                        q = mpool.tile([HALF, 2, T], dt.float32, name=f"q{cb}",
                                       tag=f"q{cb}")
                        nc.scalar.activation(out=q, in_=w, func=act.Square, bias=1.0)
                        a = mpool.tile([HALF, 2, T], dt.float32, name=f"a{cb}",
                                       tag=f"a{cb}")
                        nc.gpsimd.tensor_scalar(out=a, in0=q, scalar1=1.0,
                                                scalar2=1.0, op0=alu.mult,
                                                op1=alu.add)
                        ra = mpool.tile([HALF, 2, T], dt.float32, name=f"ra{cb}",
                                        tag=f"ra{cb}")
                        nc.vector.reciprocal_approx_fast(out=ra, in_=a)
                        ot = otpool.tile([HALF, 2, T], dt.float32, name=f"ot{cb}",
                                         tag=f"ot{cb}")
                        # ot = (-2*ra + 1) * zt in one custom-DVE pass
                        nc.vector.affine_mul_reduce(
                            out=ot, accum_out=None,
                            in0=ra, in1=zt, scale=-2.0, bias=1.0)
                        nc.sync.dma_start(out=yo[p, cb], in_=ot)

            def emit_stats(swsum, swsq, nw2):
                # ---- wave statistics (deferred one wave so the PE never
                # stalls waiting for the accumulators) ----
                # group-sum via 0/1 matmuls; single f32r precision is plenty.
                sp = spsum.tile([HALF, 2, 2 * PW], dt.float32, name="sp",
                                tag="sp")
                for cb in range(2):
                    sumh = statp.tile([HALF, 2 * PW], dt.float32r,
                                      name=f"sumh{cb}", tag=f"sumh{cb}")
                    nc.vector.tensor_copy(sumh, swsum[cb])
                    sqh = statp.tile([HALF, 2 * PW], dt.float32r,
                                     name=f"sqh{cb}", tag=f"sqh{cb}")
                    nc.vector.tensor_copy(sqh, swsq[cb])
                    nc.tensor.matmul(sp[:, 0, :], gmat_s[:, cb, :], sumh,
                                     start=(cb == 0), stop=False)
                    nc.tensor.matmul(sp[:, 1, :], gmat_s[:, cb, :], sqh,
                                     start=False, stop=(cb == 1))

                R = statp.tile([HALF, 2, 2 * PW], dt.float32, name="R", tag="R")
                nc.vector.memset(R, 0.0)
                negmu = R[0:G, 0, :nw2]
                nc.vector.tensor_scalar(out=negmu, in0=sp[0:G, 0, :nw2],
                                        scalar1=-inv_n, scalar2=None, op0=alu.mult)
                m2e = statp.tile([G, 2 * PW], dt.float32, name="m2e", tag="m2e")
                nc.vector.tensor_scalar(out=m2e[:, :nw2], in0=sp[0:G, 1, :nw2],
                                        scalar1=inv_n, scalar2=EPS,
                                        op0=alu.mult, op1=alu.add)
                ve = statp.tile([G, 2 * PW], dt.float32, name="ve", tag="ve")
                nc.vector.tensor_tensor(out=ve[:, :nw2], in0=negmu, in1=negmu,
                                        op=alu.mult)
                nc.vector.tensor_tensor(out=ve[:, :nw2], in0=m2e[:, :nw2],
                                        in1=ve[:, :nw2], op=alu.subtract)
                # rsqrt via bit trick + Newton (all on DVE, tiny tiles)
                yi = statp.tile([G, 2 * PW], dt.int32, name="yi", tag="yi")
                nc.vector.tensor_scalar(out=yi[:, :nw2],
                                        in0=ve[:, :nw2].bitcast(dt.int32),
                                        scalar1=1, scalar2=None,
                                        op0=alu.arith_shift_right)
                nc.vector.tensor_tensor(out=yi[:, :nw2], in0=magic_s[:, :nw2],
                                        in1=yi[:, :nw2], op=alu.subtract)
                yf = yi.bitcast(dt.float32)
                xh2 = statp.tile([G, 2 * PW], dt.float32, name="xh2", tag="xh2")
                nc.vector.tensor_scalar(out=xh2[:, :nw2], in0=ve[:, :nw2],
                                        scalar1=0.5, scalar2=None, op0=alu.mult)
                aa = statp.tile([G, 2 * PW], dt.float32, name="aa", tag="aa")
                dd = statp.tile([G, 2 * PW], dt.float32, name="dd", tag="dd")
                for it in range(3):
                    nc.vector.tensor_tensor(out=aa[:, :nw2], in0=yf[:, :nw2],
                                            in1=yf[:, :nw2], op=alu.mult)
                    nc.vector.tensor_tensor(out=aa[:, :nw2], in0=xh2[:, :nw2],
                                            in1=aa[:, :nw2], op=alu.mult)
                    nc.vector.tensor_scalar(out=dd[:, :nw2], in0=aa[:, :nw2],
                                            scalar1=-1.0, scalar2=1.5,
                                            op0=alu.mult, op1=alu.add)
                    outp = R[0:G, 1, :nw2] if it == 2 else yf[:, :nw2]
                    nc.vector.tensor_tensor(out=outp, in0=yf[:, :nw2],
                                            in1=dd[:, :nw2], op=alu.mult)

                Rf = R.rearrange("p a b -> p (a b)")
                Rh = statp.tile([HALF, 2 * 2 * PW], dt.float32r, name="Rh", tag="Rh")
                nc.vector.tensor_copy(Rh, Rf)
                scols = []
                tcols = []
                bpt = bpsum.tile([HALF, 2, 2 * 2 * PW], dt.float32, name="bp",
                                 tag="bp")
                for cb in range(2):
                    nc.tensor.matmul(bpt[:, cb, :], amat_s[:, cb, :], Rh,
                                     start=(cb == 0), stop=(cb == 1))
                for cb in range(2):
                    bp = bpt[:, cb, :].rearrange("p (a b) -> p a b", a=2)
                    scol = stp.tile([HALF, 2 * PW], dt.float32, name=f"scol{cb}",
                                    tag=f"scol{cb}")
                    nc.vector.tensor_scalar(out=scol[:, :nw2], in0=bp[:, 1, :nw2],
                                            scalar1=gamma_s[:, cb:cb + 1],
                                            scalar2=None, op0=alu.mult)
                    tcol = stp.tile([HALF, 2 * PW], dt.float32, name=f"tcol{cb}",
                                    tag=f"tcol{cb}")
                    nc.vector.tensor_tensor(out=tcol[:, :nw2], in0=bp[:, 0, :nw2],
                                            in1=scol[:, :nw2], op=alu.mult)
                    nc.vector.tensor_scalar(out=tcol[:, :nw2], in0=tcol[:, :nw2],
                                            scalar1=beta_s[:, cb:cb + 1],
                                            scalar2=None, op0=alu.add)
                    scols.append(scol)
                    tcols.append(tcol)
                return scols, tcols

            def flush_prev(wave_state):
                wave_pairs, swsum, swsq, nw2 = wave_state
                scols, tcols = emit_stats(swsum, swsq, nw2)
                emit_mish((wave_pairs, scols, tcols))

            # tapered wave sizes: full waves first, then shrink so the tail
            # after the last conv wave is a tiny mish flush
            wave_sizes = []
            rem = NP
            while rem > PW:
                wave_sizes.append(PW)
                rem -= PW
            while rem > 0:
                step = max(1, rem // 2)
                wave_sizes.append(step)
                rem -= step

            p0 = 0
            for wsz in wave_sizes:
                p1 = p0 + wsz
                nw2 = 2 * wsz
                swsum = [swpool.tile([HALF, 2 * PW], dt.float32, name=f"sws{cb}",
                                     tag=f"sws{cb}") for cb in range(2)]
                swsq = [swpool.tile([HALF, 2 * PW], dt.float32, name=f"swq{cb}",
                                    tag=f"swq{cb}") for cb in range(2)]
                if nw2 < 2 * PW:
                    for cb in range(2):
                        nc.vector.memset(swsum[cb], 0.0)
                        nc.scalar.memzero(swsq[cb])
                wave_pairs = []
                for p in range(p0, p1):
                    iw0 = 2 * (p - p0)
                    xt_h = []
                    for cib in range(2):
                        th = xpool.tile([HALF, 2, TP], dt.float32r,
                                        name=f"xh{cib}", tag=f"xh{cib}")
                        nc.sync.dma_start(out=th, in_=xh[p, cib].rearrange(
                            "p (s t) -> p s t", s=2))
                        xt_h.append(th)
                    ybs = []
                    for cb in range(2):
                        cp = cpsum.tile([HALF, 2, T], dt.float32, name=f"cp{cb}",
                                        tag=f"cp{cb}")
                        # one accumulation group covering both samples in the
                        # bank: only the very first matmul carries start=True
                        # (it clears has_written for the whole bank; later
                        # first-touches overwrite, repeats accumulate).
                        first = True
                        for cib in range(2):
                            for k in range(KS):
                                for s in range(2):
                                    group_last = (s == 1 and cib == 1
                                                  and k == KS - 1)
                                    nc.tensor.matmul(
                                        cp[:, s, :], wsb_h[:, cb, cib, k, :],
                                        xt_h[cib][:, s, k:k + T], start=first,
                                        stop=group_last)
                                    first = False
                        yb = ybpool.tile([HALF, 2, T], dt.float32, name=f"yb{cb}",
                                         tag=f"yb{cb}")
                        for s in range(2):
                            # bias + PSUM->SBUF eviction + sum stat on DVE
                            # (Pool/GPSIMD cannot access PSUM on TRN2)
                            nc.vector.tensor_scalar(
                                out=yb[:, s, :], in0=cp[:, s, :],
                                scalar1=bias_s[:, cb:cb + 1], scalar2=0.0,
                                op0=alu.add, op1=alu.add,
                                accum_out=swsum[cb][:, iw0 + s:iw0 + s + 1])
                            y2 = y2pool.tile([HALF, T], dt.float32, name="y2",
                                             tag="y2")
                            # sumsq of (y+bias): Square reads PSUM directly
                            # with the bias folded into the ACT affine.
                            nc.scalar.activation(
                                out=y2, in_=cp[:, s, :], func=act.Square,
                                bias=bias_s[:, cb:cb + 1],
                                accum_out=swsq[cb][:, iw0 + s:iw0 + s + 1])
                        ybs.append(yb)
                    wave_pairs.append((p, ybs, iw0))

                if prev_wave is not None:
                    flush_prev(prev_wave)
                prev_wave = (wave_pairs, swsum, swsq, nw2)
                p0 = p1

            flush_prev(prev_wave)

    nc.finalize()
    return nc


def kernel(x, use_expert_i, W, b, gamma, beta):
    global LAST_EXEC_NS, LAST_RESULTS
    from concourse.bass_utils import run_bass_kernel_spmd

    if TRACE:
        _install_trace_hook()

    x = np.asarray(x, dtype=np.float32)
    u = np.asarray(use_expert_i).astype(np.int64)
    W = np.asarray(W, dtype=np.float32)
    b = np.asarray(b, dtype=np.float32)
    gamma = np.asarray(gamma, dtype=np.float32)
    beta = np.asarray(beta, dtype=np.float32)

    counts = np.bincount(u, minlength=E)
    n_max = max(int(counts.max()), 2)
    NP = (n_max + 1) // 2

    key = NP
    if key not in _prog_cache:
        _prog_cache[key] = _build_program(NP)
    nc = _prog_cache[key]

    # ---- host-side dispatch (the sharding step) ----
    idx_lists = []
    in_maps = []
    # group-indicator matrices, shared across cores
    gmat = np.zeros((2, HALF, HALF), np.float32)
    amat = np.zeros((2, HALF, HALF), np.float32)
    for cb in range(2):
        for p in range(HALF):
            g = cb * (G // 2) + p // GRP
            gmat[cb, p, g] = 1.0
            amat[cb, g, p] = 1.0

    for e in range(E):
        idx = np.nonzero(u == e)[0]
        pad_to = NP * 2
        if len(idx) == 0:
            padded = np.zeros(pad_to, np.int64)
        else:
            padded = np.concatenate([idx, np.full(pad_to - len(idx), idx[0])])
        idx_lists.append((idx, padded))

        xs = x[padded]  # [2*NP, C, T]
        # padded layout [NP, ci_blk, 128, 2*(T+4)] with zero halo columns
        xpad = np.zeros((NP, 2, HALF, 2, TP), np.float32)
        xv = xs.reshape(NP, 2, 2, HALF, T).transpose(0, 2, 3, 1, 4)
        xpad[:, :, :, :, 2:2 + T] = xv

        # weights [co_blk, ci_blk, ci, k, co]
        we = W[e].reshape(2, HALF, 2, HALF, KS).transpose(0, 2, 3, 4, 1)
        we = np.ascontiguousarray(we)

        in_maps.append({
            "xh": xpad.reshape(NP, 2, HALF, 2 * TP),
            "wh": we,
            "bias2": np.ascontiguousarray(b[e].reshape(2, HALF).T),
            "gamma2": np.ascontiguousarray(gamma[e].reshape(2, HALF).T),
            "beta2": np.ascontiguousarray(beta[e].reshape(2, HALF).T),
            "gmat": gmat,
            "amat": amat,
        })

    res = run_bass_kernel_spmd(nc, in_maps, list(range(E)), trace=TRACE)
    LAST_EXEC_NS = res.exec_time_ns
    LAST_RESULTS = res

    out = np.empty((B, C, T), np.float32)
    for e in range(E):
        idx, padded = idx_lists[e]
        yo = res.results[e]["yo"]  # [NP, 2, 128, 2, T]
        ye = yo.transpose(0, 3, 1, 2, 4).reshape(NP * 2, C, T)
        if len(idx):
            out[idx] = ye[: len(idx)]
    return out
